# revision 30
# baseline (speedup 1.0000x reference)
"""Trainium2 Bass kernel for nn_Encoder_83992380441041 (causal linear attention
encoder, last-position readout).

Math (per segment b of T tokens):
    yn   = LayerNorm(x_b) * gamma + beta          (beta == 0 here)
    K    = phi(yn @ Wk.T); V = yn @ Wv.T; q = phi(yn[T-1] @ Wq.T)
    out  = q @ (K.T V) / (q . sum_t K_t + eps)    [only last position matters]
with phi(a) = elu(a)+1 = min(exp(a),1) + relu(a).

Host folds (all O(N*D) or O(D*F), cheap on CPU):
  * gamma and LN centering into the weights: x @ (W - 1 s~/d) = (x - mu 1) @ W.
  * the LN scale r = 1/sqrt(var+eps) into x itself: ship xs = fp16(r * x).
    Then G = xs @ W' is exactly the phi/V pre-activation; no per-token scale
    or bn_stats on device at all.
Device (per core: 2048 tokens = 8 segments x 256, fp16 data path):
  * xsT [128d x 2048tok] arrives via DMA-transpose (2-byte xbar path).
  * G tile [128tok x 128] = matmul(lhsT=xsT tile, rhs=[Wk'|Wv']).
  * phi: e=Exp(Gk) [ACT], s=relu(Gk) [DVE max], K=min(e,1)+s [DVE stt];
    V copied PSUM->SBUF with ones column appended.
  * S|Z per segment: accumulate matmul(lhsT=K tile, rhs=[V|1]) -> [64 x 65],
    two segments per PSUM bank via tile_position.
  * readout: nd[:,s] = matmul(lhsT=ssb_pair, rhs=qstack col) = [num|q.Z],
    tiny transpose, out = num/(den+eps).

Sharding: data-parallel over segments. 64 segments -> 8 cores x 8 segments.
"""

import os as _os

import numpy as np

import concourse.bass as bass
import concourse.tile as tile
from concourse import mybir
import concourse.bass_utils as _bass_utils
from concourse.bass_utils import run_bass_kernel_spmd
from concourse.vector_clock import ScopedClock
import bass_rust

EPS_LN = 1e-5
EPS_DEN = 1e-5

F32 = mybir.dt.float32
F16 = mybir.dt.float16
AF = mybir.ActivationFunctionType
ALU = mybir.AluOpType

N_CORES = 8


def _patched_drain_and_barrier(self, tick_clock, wait_clock):
    # Stock TileContext exit puts one sem-wait per outstanding proc on a
    # single InstDrain; walrus in this container caps sync waits per
    # instruction. Split them across a chain of drains on the same engine
    # (program order preserved => equivalent).
    nc = self.nc
    drain_inst = nc.sync.drain()
    wait_clock.add_sem_waits(
        drain_inst.ins, ScopedClock({None: tick_clock.global_clock})
    )
    si = drain_inst.ins.sync_info
    if si is not None and si.on_wait is not None and len(si.on_wait) > 1:
        waits = list(si.on_wait)
        si.on_wait = waits[:1]
        for w in waits[1:]:
            d2 = nc.sync.drain()
            si2 = d2.ins.sync_info
            if si2 is None:
                d2.ins.sync_info = bass_rust.SyncInfo(on_wait=[w], on_update=[])
            else:
                si2.on_wait = [w]
    nc.all_engine_barrier()
    assert self.sems is not None
    popped = nc._tile_sem_poison_stack.pop()
    assert popped is self._sem_poison
    nc.clear_and_free_semaphores(list(self.sems.allocated().values()))


tile.TileContext._drain_and_barrier = _patched_drain_and_barrier

_orig_commit = tile.TileContext._commit_instruction
_wsplit_counter = [0]


def _patched_commit_instruction(self, inst, lazy_reg_writes: bool = True):
    # Enforce the per-instruction sync-wait capacity of the walrus in this
    # container (1 for regular instructions, 2 for EventSemaphore) by
    # spilling excess waits onto same-engine NOPs committed just before.
    si = getattr(inst, "sync_info", None)
    if si is not None and si.on_wait:
        cap = 2 if isinstance(inst, mybir.InstEventSemaphore) else 1
        if len(si.on_wait) > cap:
            waits = list(si.on_wait)
            si.on_wait = waits[:cap]
            for w in waits[cap:]:
                _wsplit_counter[0] += 1
                nop = mybir.InstNoOp(
                    name=f"wsplit-{_wsplit_counter[0]}",
                    sync_info=mybir.SyncInfo(on_wait=[w], on_update=[]),
                    bass_nofuse=True,
                    engine=inst.engine,
                )
                _orig_commit(self, nop, lazy_reg_writes=False)
    return _orig_commit(self, inst, lazy_reg_writes=lazy_reg_writes)


tile.TileContext._commit_instruction = _patched_commit_instruction


# Optional extra walrus args (experiment hook), e.g.
#   KERNEL_WALRUS_ARGS="--max-sem-num=32"
_EXTRA_WALRUS_ARGS = [a for a in _os.environ.get("KERNEL_WALRUS_ARGS", "").split() if a]
if _EXTRA_WALRUS_ARGS:
    _orig_bvao = _bass_utils.bir_verify_and_optimise

    def _patched_bvao(tmpdir, inp="bir.json", outp="file.neff", arch=None, *, dve_root=None):
        import subprocess as _sp

        orig_run = _bass_utils.run_command

        def run_with_extra(cmd, **kw):
            return orig_run(list(cmd) + _EXTRA_WALRUS_ARGS, **kw)

        _bass_utils.run_command = run_with_extra
        try:
            return _orig_bvao(tmpdir, inp, outp, arch, dve_root=dve_root)
        finally:
            _bass_utils.run_command = orig_run

    _bass_utils.bir_verify_and_optimise = _patched_bvao


def _build(n_tok: int, n_seg: int, d: int, f: int):
    """Per-core program. Inputs: xs [n_tok,d] fp16 (pre-scaled by r*gamma-fold);
    wkvq [d, 3f] fp16 = [Wk'|Wv'|Wq'] (columns); ident [f+1, f+1] fp16.
    Output: z [n_seg, f] fp32."""
    P = 128
    assert d == P and n_tok % P == 0
    n_tiles = n_tok // P
    t_seg = n_tok // n_seg
    tiles_per_seg = t_seg // P
    assert tiles_per_seg * n_seg == n_tiles
    f2 = 2 * f
    assert f2 == P
    fp1 = f + 1
    B = 4                      # tiles per block (PSUM bank / phi batch)
    n_blk = n_tiles // B
    n_pair = n_seg // 2

    nc = bass.Bass()
    xs_d = nc.declare_dram_parameter("xs", [n_tok, d], F16, isOutput=False)
    wkvq_d = nc.declare_dram_parameter("wkvq", [d, 3 * f], F16, isOutput=False)
    z_d = nc.declare_dram_parameter("z", [n_seg, f], F32, isOutput=True)

    with tile.TileContext(nc) as tc:
        with (
            tc.tile_pool(name="singles", bufs=1) as singles,
            tc.tile_pool(name="phi", bufs=3) as phip,
            tc.tile_pool(name="ssb", bufs=4) as ssbp,
            tc.tile_pool(name="fin", bufs=1) as finp,
            tc.tile_pool(name="psG", bufs=2, space="PSUM") as psG,
            tc.tile_pool(name="psS", bufs=2, space="PSUM") as psS,
            tc.tile_pool(name="psM", bufs=1, space="PSUM") as psM,
        ):
            # --- persistent buffers ---
            xsT = singles.tile([P, n_tok], F16)
            wkvq = singles.tile([P, 3 * f], F16)
            kbig = singles.tile([P, n_tiles * f], F16)
            v1big = singles.tile([P, n_tiles * fp1], F16)
            jbig = singles.tile([P, 512], F16)

            # --- DMA triggers.  wkvq goes first on gpsimd (SWDGE); the two
            # x transposes (committed after it) each pick up one
            # transpose-vs-DMA serialization edge on it, which only costs
            # ~2.3us — less than a second HWDGE trigger on either engine.
            half = n_tok // 2
            nc.gpsimd.dma_start(out=wkvq[:], in_=wkvq_d[:])
            nc.scalar.dma_start(out=xsT[:, 0:half], in_=xs_d[0:half, :], transpose=True)
            nc.sync.dma_start(out=xsT[:, half:], in_=xs_d[half:, :], transpose=True)

            # --- PE warm-up junk: no data deps, fills the DMA wait and
            # releases the HAM clock throttle before the real matmuls.
            nc.vector.memset(jbig[:], 0.0)
            for _ in range(4):
                jps = psM.tile([P, 512], F32, tag="m")
                nc.tensor.matmul(
                    jps[:], lhsT=jbig[:, 0:P], rhs=jbig[:],
                    start=True, stop=True, skip_group_check=True,
                )

            # ones columns of [V|1] (stride fp1 within v1big)
            nc.vector.memset(v1big[:, f::fp1], 1.0)

            # qpad: 4 zero-padded copies of qstack, one per segment pair —
            # pair p's slice has only columns 2p,2p+1 nonzero, so the four
            # readout matmuls can accumulate into one [n_seg, f+1] group.
            qpad = finp.tile([P, n_pair * n_seg], F16)
            zden = finp.tile([n_seg, 1], F32)
            zout = finp.tile([n_seg, f], F32)
            eq = finp.tile([P, n_seg], F16)
            sq = finp.tile([P, n_seg], F16)
            q2 = finp.tile([P, n_seg], F16)

            wkv_s = wkvq[:, 0:f2]
            wq_s = wkvq[:, f2:3 * f]

            # --- per-block (4 tiles = 1 segment pair): G -> batched phi /
            # V copy -> S -> ssb copy.  V/ssb copies alternate ACT and DVE
            # to balance the two elementwise engines.
            ssbs = []
            for b in range(n_blk):
                b0 = b * B
                gps = psG.tile([P, B * P], F32)
                for j in range(B):
                    nc.tensor.matmul(
                        gps[:, j * P:(j + 1) * P],
                        lhsT=xsT[:, (b0 + j) * P:(b0 + j + 1) * P],
                        rhs=wkv_s,
                        start=True, stop=True, skip_group_check=True,
                    )
                blk = gps[:].rearrange("p (j c) -> p j c", c=P)
                gk = blk[:, :, 0:f]
                gv = blk[:, :, f:f2]
                e_t = phip.tile([P, B * f], F16, tag="e")
                nc.scalar.activation(out=e_t[:], in_=gk, func=AF.Exp)
                s_t = phip.tile([P, B * f], F16, tag="s")
                nc.vector.tensor_scalar_max(out=s_t[:], in0=gk, scalar1=0.0)
                nc.vector.scalar_tensor_tensor(
                    out=kbig[:, b0 * f:(b0 + B) * f], in0=e_t[:],
                    scalar=1.0, in1=s_t[:],
                    op0=ALU.min, op1=ALU.add,
                )
                vdst = v1big[:, b0 * fp1:(b0 + B) * fp1].rearrange(
                    "p (j c) -> p j c", c=fp1
                )[:, :, 0:f]
                if b >= 2:
                    nc.scalar.activation(out=vdst, in_=gv, func=AF.Copy)
                else:
                    nc.vector.tensor_copy(out=vdst, in_=gv)

                # S|Z for the block's two segments
                assert B == 2 * tiles_per_seg
                sps = psS.tile([P, fp1], F32)
                for hh in range(2):
                    s = 2 * b + hh
                    for j in range(tiles_per_seg):
                        n = s * tiles_per_seg + j
                        nc.tensor.matmul(
                            sps[hh * f:(hh + 1) * f, :],
                            lhsT=kbig[:, n * f:(n + 1) * f],
                            rhs=v1big[:, n * fp1:(n + 1) * fp1],
                            start=(j == 0), stop=(j == tiles_per_seg - 1),
                            skip_group_check=True,
                            tile_position=(0, hh * f),
                        )
                ssb = ssbp.tile([P, fp1], F16)
                if b >= 2:
                    nc.scalar.activation(out=ssb[:], in_=sps[:], func=AF.Copy)
                else:
                    nc.vector.tensor_copy(out=ssb[:], in_=sps[:])
                ssbs.append(ssb)

            # --- q path: last token of each segment, both PE-halves at once
            xq = xsT[:, t_seg - 1::t_seg]
            qc = psM.tile([P, n_seg], F32, tag="m")
            nc.tensor.matmul(
                qc[0:f, :], lhsT=wq_s, rhs=xq,
                start=True, stop=True, skip_group_check=True,
            )
            nc.tensor.matmul(
                qc[f:f2, :], lhsT=wq_s, rhs=xq,
                start=True, stop=True, skip_group_check=True,
                tile_position=(0, f),
            )
            nc.scalar.activation(out=eq[:], in_=qc[:], func=AF.Exp)
            nc.vector.tensor_scalar_max(out=sq[:], in0=qc[:], scalar1=0.0)
            nc.vector.scalar_tensor_tensor(
                out=q2[:], in0=eq[:], scalar=1.0, in1=sq[:],
                op0=ALU.min, op1=ALU.add,
            )
            # pair p's even segment lives at global col 10p, odd at 10p+1
            nc.vector.memset(qpad[:], 0.0)
            nc.vector.tensor_copy(
                out=qpad[0:f, 0::2 + n_seg], in_=q2[0:f, 0:n_seg:2]
            )
            nc.vector.tensor_copy(
                out=qpad[f:f2, 1::2 + n_seg], in_=q2[f:f2, 1:n_seg:2]
            )

            # --- readout, accumulated across pairs:
            #   nd[s, :] = qhat_s^T @ ssb_pair(s) = [num_s | q_s.Z_s]
            nd = psM.tile([n_seg, fp1], F32, tag="nd")
            for p in range(n_pair):
                nc.tensor.matmul(
                    nd[:], lhsT=qpad[:, p * n_seg:(p + 1) * n_seg],
                    rhs=ssbs[p],
                    start=(p == 0), stop=(p == n_pair - 1),
                    skip_group_check=True,
                )
            nc.vector.tensor_scalar_add(
                out=zden[:], in0=nd[:, f:fp1], scalar1=EPS_DEN
            )
            nc.vector.reciprocal(out=zden[:], in_=zden[:])
            nc.vector.tensor_scalar_mul(
                out=zout[:], in0=nd[:, 0:f], scalar1=zden[:]
            )
            nc.sync.dma_start(out=z_d[:], in_=zout[:])

    return nc


def _prep(inputs):
    x = np.asarray(inputs["x"], dtype=np.float32)
    batch = np.asarray(inputs["batch"]).astype(np.int64)
    gamma = np.asarray(inputs["gamma"], dtype=np.float32)
    beta = np.asarray(inputs["beta"], dtype=np.float32)
    wk = np.asarray(inputs["Wk"], dtype=np.float32)
    wq = np.asarray(inputs["Wq"], dtype=np.float32)
    wv = np.asarray(inputs["Wv"], dtype=np.float32)
    n_batches = int(np.asarray(inputs["n_batches"]))

    n, d = x.shape
    f = wk.shape[0]
    t_seg = n // n_batches
    counts = np.bincount(batch, minlength=n_batches)
    if not (np.all(counts == t_seg) and np.all(np.diff(batch) >= 0)):
        raise NotImplementedError("kernel specialized for equal sorted segments")
    if np.any(beta != 0.0):
        raise NotImplementedError("kernel specialized for beta == 0")

    # LN scale folded into x on the host; centering+gamma folded into W.
    xd = x.astype(np.float64)
    var = xd.var(axis=1)
    r = 1.0 / np.sqrt(var + EPS_LN)
    xs = np.ascontiguousarray((xd * r[:, None]).astype(np.float16))

    wkg = (wk * gamma[None, :]).astype(np.float64)
    wvg = (wv * gamma[None, :]).astype(np.float64)
    wqg = (wq * gamma[None, :]).astype(np.float64)
    wall = np.concatenate([wkg, wvg, wqg], axis=0).T        # [d, 3f]
    wall = wall - wall.sum(axis=0, keepdims=True) / d
    wall = np.ascontiguousarray(wall.astype(np.float16))    # [d, 3f]

    return xs, wall, n, d, f, n_batches, t_seg


def _run(inputs, trace=False):
    xs, wall, n, d, f, n_batches, t_seg = _prep(inputs)

    segs_per_core = n_batches // N_CORES
    tok_per_core = segs_per_core * t_seg
    nc = _build(tok_per_core, segs_per_core, d, f)

    in_maps = []
    for c in range(N_CORES):
        m = {
            "xs": np.ascontiguousarray(xs[c * tok_per_core:(c + 1) * tok_per_core]),
            "wkvq": wall,
        }
        in_maps.append(m)

    res = run_bass_kernel_spmd(nc, in_maps, list(range(N_CORES)), trace=trace)
    z = np.concatenate([res.results[c]["z"] for c in range(N_CORES)], axis=0)
    return z, res


def kernel(**inputs) -> np.ndarray:
    z, _ = _run(inputs, trace=False)
    return z


# revision 34
# speedup vs baseline: 1.1102x; 1.1102x over previous
"""Trainium2 Bass kernel for nn_Encoder_83992380441041 (causal linear attention
encoder, last-position readout).

Math (per segment b of T tokens):
    yn   = LayerNorm(x_b) * gamma + beta          (beta == 0 here)
    K    = phi(yn @ Wk.T); V = yn @ Wv.T; q = phi(yn[T-1] @ Wq.T)
    out  = q @ (K.T V) / (q . sum_t K_t + eps)    [only last position matters]
with phi(a) = elu(a)+1 = min(exp(a),1) + relu(a).

Host folds (all O(N*D) or O(D*F), cheap on CPU):
  * gamma and LN centering into the weights: x @ (W - 1 s~/d) = (x - mu 1) @ W.
  * the LN scale r = 1/sqrt(var+eps) into x itself: ship xs = fp16(r * x).
    Then G = xs @ W' is exactly the phi/V pre-activation; no per-token scale
    or bn_stats on device at all.
Device (per core: 2048 tokens = 8 segments x 256, fp16 data path):
  * xsT [128d x 2048tok] arrives via DMA-transpose (2-byte xbar path).
  * G tile [128tok x 128] = matmul(lhsT=xsT tile, rhs=[Wk'|Wv']).
  * phi: e=Exp(Gk) [ACT], s=relu(Gk) [DVE max], K=min(e,1)+s [DVE stt];
    V copied PSUM->SBUF with ones column appended.
  * S|Z per segment: accumulate matmul(lhsT=K tile, rhs=[V|1]) -> [64 x 65],
    two segments per PSUM bank via tile_position.
  * readout: nd[:,s] = matmul(lhsT=ssb_pair, rhs=qstack col) = [num|q.Z],
    tiny transpose, out = num/(den+eps).

Sharding: data-parallel over segments. 64 segments -> 8 cores x 8 segments.
"""

import os as _os

import numpy as np

import concourse.bass as bass
import concourse.tile as tile
from concourse import mybir
import concourse.bass_utils as _bass_utils
from concourse.bass_utils import run_bass_kernel_spmd
from concourse.vector_clock import ScopedClock
import bass_rust

EPS_LN = 1e-5
EPS_DEN = 1e-5

F32 = mybir.dt.float32
F16 = mybir.dt.float16
AF = mybir.ActivationFunctionType
ALU = mybir.AluOpType

N_CORES = 8


def _patched_drain_and_barrier(self, tick_clock, wait_clock):
    # Stock TileContext exit puts one sem-wait per outstanding proc on a
    # single InstDrain; walrus in this container caps sync waits per
    # instruction. Split them across a chain of drains on the same engine
    # (program order preserved => equivalent).
    nc = self.nc
    drain_inst = nc.sync.drain()
    wait_clock.add_sem_waits(
        drain_inst.ins, ScopedClock({None: tick_clock.global_clock})
    )
    si = drain_inst.ins.sync_info
    if si is not None and si.on_wait is not None and len(si.on_wait) > 1:
        waits = list(si.on_wait)
        si.on_wait = waits[:1]
        for w in waits[1:]:
            d2 = nc.sync.drain()
            si2 = d2.ins.sync_info
            if si2 is None:
                d2.ins.sync_info = bass_rust.SyncInfo(on_wait=[w], on_update=[])
            else:
                si2.on_wait = [w]
    nc.all_engine_barrier()
    assert self.sems is not None
    popped = nc._tile_sem_poison_stack.pop()
    assert popped is self._sem_poison
    nc.clear_and_free_semaphores(list(self.sems.allocated().values()))


tile.TileContext._drain_and_barrier = _patched_drain_and_barrier

_orig_commit = tile.TileContext._commit_instruction
_wsplit_counter = [0]


def _patched_commit_instruction(self, inst, lazy_reg_writes: bool = True):
    # Enforce the per-instruction sync-wait capacity of the walrus in this
    # container (1 for regular instructions, 2 for EventSemaphore) by
    # spilling excess waits onto same-engine NOPs committed just before.
    si = getattr(inst, "sync_info", None)
    if si is not None and si.on_wait:
        cap = 2 if isinstance(inst, mybir.InstEventSemaphore) else 1
        if len(si.on_wait) > cap:
            waits = list(si.on_wait)
            si.on_wait = waits[:cap]
            for w in waits[cap:]:
                _wsplit_counter[0] += 1
                nop = mybir.InstNoOp(
                    name=f"wsplit-{_wsplit_counter[0]}",
                    sync_info=mybir.SyncInfo(on_wait=[w], on_update=[]),
                    bass_nofuse=True,
                    engine=inst.engine,
                )
                _orig_commit(self, nop, lazy_reg_writes=False)
    return _orig_commit(self, inst, lazy_reg_writes=lazy_reg_writes)


tile.TileContext._commit_instruction = _patched_commit_instruction


# Optional extra walrus args (experiment hook), e.g.
#   KERNEL_WALRUS_ARGS="--max-sem-num=32"
_EXTRA_WALRUS_ARGS = [a for a in _os.environ.get("KERNEL_WALRUS_ARGS", "").split() if a]
if _EXTRA_WALRUS_ARGS:
    _orig_bvao = _bass_utils.bir_verify_and_optimise

    def _patched_bvao(tmpdir, inp="bir.json", outp="file.neff", arch=None, *, dve_root=None):
        import subprocess as _sp

        orig_run = _bass_utils.run_command

        def run_with_extra(cmd, **kw):
            return orig_run(list(cmd) + _EXTRA_WALRUS_ARGS, **kw)

        _bass_utils.run_command = run_with_extra
        try:
            return _orig_bvao(tmpdir, inp, outp, arch, dve_root=dve_root)
        finally:
            _bass_utils.run_command = orig_run

    _bass_utils.bir_verify_and_optimise = _patched_bvao


def _build(n_tok: int, n_seg: int, d: int, f: int):
    """Per-core program. Inputs: xs [n_tok,d] fp16 (pre-scaled by r*gamma-fold);
    wkvq [d, 3f] fp16 = [Wk'|Wv'|Wq'] (columns); ident [f+1, f+1] fp16.
    Output: z [n_seg, f] fp32."""
    P = 128
    assert d == P and n_tok % P == 0
    n_tiles = n_tok // P
    t_seg = n_tok // n_seg
    tiles_per_seg = t_seg // P
    assert tiles_per_seg * n_seg == n_tiles
    f2 = 2 * f
    assert f2 == P
    fp1 = f + 1
    B = 4                      # tiles per block (PSUM bank / phi batch)
    n_blk = n_tiles // B
    n_pair = n_seg // 2

    nc = bass.Bass()
    xs_d = nc.declare_dram_parameter("xs", [n_tok, d], F16, isOutput=False)
    # wpackT rows 0..3f-1 = [Wk'|Wv'|Wq'] columns; DMA-transposed on load.
    wpackT_d = nc.declare_dram_parameter("wpackT", [3 * f, d], F16, isOutput=False)
    z_d = nc.declare_dram_parameter("z", [n_seg, f], F32, isOutput=True)

    with tile.TileContext(nc) as tc:
        with (
            tc.tile_pool(name="singles", bufs=1) as singles,
            tc.tile_pool(name="phi", bufs=3) as phip,
            tc.tile_pool(name="ssb", bufs=4) as ssbp,
            tc.tile_pool(name="fin", bufs=1) as finp,
            tc.tile_pool(name="psG", bufs=2, space="PSUM") as psG,
            tc.tile_pool(name="psS", bufs=2, space="PSUM") as psS,
            tc.tile_pool(name="psM", bufs=1, space="PSUM") as psM,
        ):
            # --- persistent buffers ---
            xsT = singles.tile([P, n_tok], F16)
            wkvq = singles.tile([P, 3 * f], F16)
            kbig = singles.tile([P, n_tiles * f], F16)
            v1big = singles.tile([P, n_tiles * fp1], F16)
            jbig = singles.tile([P, 512], F16)

            # --- DMA triggers.  All inputs arrive as DMA-transposes so the
            # tile framework's transpose-vs-DMA serialization edges never
            # fire.  ACT carries the first x half (its only DMA, so the act
            # table load follows right after); SP carries weights + 2nd half.
            half = n_tok // 2
            nc.scalar.dma_start(out=xsT[:, 0:half], in_=xs_d[0:half, :], transpose=True)
            nc.sync.dma_start(out=wkvq[:], in_=wpackT_d[:], transpose=True)
            nc.sync.dma_start(out=xsT[:, half:], in_=xs_d[half:, :], transpose=True)

            # --- PE warm-up junk: no data deps, fills the DMA wait and
            # releases the HAM clock throttle before the real matmuls.
            nc.vector.memset(jbig[:], 0.0)
            for _ in range(4):
                jps = psM.tile([P, 512], F32, tag="m")
                nc.tensor.matmul(
                    jps[:], lhsT=jbig[:, 0:P], rhs=jbig[:],
                    start=True, stop=True, skip_group_check=True,
                )

            # ones columns of [V|1] (stride fp1 within v1big)
            nc.vector.memset(v1big[:, f::fp1], 1.0)

            # qpad: 4 zero-padded copies of qstack, one per segment pair —
            # pair p's slice has only columns 2p,2p+1 nonzero, so the four
            # readout matmuls can accumulate into one [n_seg, f+1] group.
            qpad = finp.tile([P, n_pair * n_seg], F16)
            zden = finp.tile([n_seg, 1], F32)
            zout = finp.tile([n_seg, f], F32)
            eq = finp.tile([P, n_seg], F16)
            sq = finp.tile([P, n_seg], F16)
            q2 = finp.tile([P, n_seg], F16)

            wkv_s = wkvq[:, 0:f2]
            wq_s = wkvq[:, f2:3 * f]

            # --- per-block (4 tiles = 1 segment pair): G -> batched phi /
            # V copy -> S -> ssb copy.  V/ssb copies alternate ACT and DVE
            # to balance the two elementwise engines.
            ssbs = []
            for b in range(n_blk):
                b0 = b * B
                gps = psG.tile([P, B * P], F32)
                for j in range(B):
                    nc.tensor.matmul(
                        gps[:, j * P:(j + 1) * P],
                        lhsT=xsT[:, (b0 + j) * P:(b0 + j + 1) * P],
                        rhs=wkv_s,
                        start=True, stop=True, skip_group_check=True,
                    )
                blk = gps[:].rearrange("p (j c) -> p j c", c=P)
                gk = blk[:, :, 0:f]
                gv = blk[:, :, f:f2]
                e_t = phip.tile([P, B * f], F16, tag="e")
                nc.scalar.activation(out=e_t[:], in_=gk, func=AF.Exp)
                s_t = phip.tile([P, B * f], F16, tag="s")
                nc.vector.tensor_scalar_max(out=s_t[:], in0=gk, scalar1=0.0)
                nc.vector.scalar_tensor_tensor(
                    out=kbig[:, b0 * f:(b0 + B) * f], in0=e_t[:],
                    scalar=1.0, in1=s_t[:],
                    op0=ALU.min, op1=ALU.add,
                )
                vdst = v1big[:, b0 * fp1:(b0 + B) * fp1].rearrange(
                    "p (j c) -> p j c", c=fp1
                )[:, :, 0:f]
                if b >= 2:
                    nc.scalar.activation(out=vdst, in_=gv, func=AF.Copy)
                else:
                    nc.vector.tensor_copy(out=vdst, in_=gv)

                # S|Z for the block's two segments
                assert B == 2 * tiles_per_seg
                sps = psS.tile([P, fp1], F32)
                for hh in range(2):
                    s = 2 * b + hh
                    for j in range(tiles_per_seg):
                        n = s * tiles_per_seg + j
                        nc.tensor.matmul(
                            sps[hh * f:(hh + 1) * f, :],
                            lhsT=kbig[:, n * f:(n + 1) * f],
                            rhs=v1big[:, n * fp1:(n + 1) * fp1],
                            start=(j == 0), stop=(j == tiles_per_seg - 1),
                            skip_group_check=True,
                            tile_position=(0, hh * f),
                        )
                ssb = ssbp.tile([P, fp1], F16)
                if b >= 2:
                    nc.scalar.activation(out=ssb[:], in_=sps[:], func=AF.Copy)
                else:
                    nc.vector.tensor_copy(out=ssb[:], in_=sps[:])
                ssbs.append(ssb)

            # --- q path: last token of each segment, both PE-halves at once
            xq = xsT[:, t_seg - 1::t_seg]
            qc = psM.tile([P, n_seg], F32, tag="m")
            nc.tensor.matmul(
                qc[0:f, :], lhsT=wq_s, rhs=xq,
                start=True, stop=True, skip_group_check=True,
            )
            nc.tensor.matmul(
                qc[f:f2, :], lhsT=wq_s, rhs=xq,
                start=True, stop=True, skip_group_check=True,
                tile_position=(0, f),
            )
            nc.scalar.activation(out=eq[:], in_=qc[:], func=AF.Exp)
            nc.vector.tensor_scalar_max(out=sq[:], in0=qc[:], scalar1=0.0)
            nc.vector.scalar_tensor_tensor(
                out=q2[:], in0=eq[:], scalar=1.0, in1=sq[:],
                op0=ALU.min, op1=ALU.add,
            )
            # pair p's even segment lives at global col 10p, odd at 10p+1
            nc.vector.memset(qpad[:], 0.0)
            nc.vector.tensor_copy(
                out=qpad[0:f, 0::2 + n_seg], in_=q2[0:f, 0:n_seg:2]
            )
            nc.vector.tensor_copy(
                out=qpad[f:f2, 1::2 + n_seg], in_=q2[f:f2, 1:n_seg:2]
            )

            # --- readout, accumulated across pairs:
            #   nd[s, :] = qhat_s^T @ ssb_pair(s) = [num_s | q_s.Z_s]
            nd = psM.tile([n_seg, fp1], F32, tag="nd")
            for p in range(n_pair):
                nc.tensor.matmul(
                    nd[:], lhsT=qpad[:, p * n_seg:(p + 1) * n_seg],
                    rhs=ssbs[p],
                    start=(p == 0), stop=(p == n_pair - 1),
                    skip_group_check=True,
                )
            nc.vector.tensor_scalar_add(
                out=zden[:], in0=nd[:, f:fp1], scalar1=EPS_DEN
            )
            nc.vector.reciprocal(out=zden[:], in_=zden[:])
            nc.vector.tensor_scalar_mul(
                out=zout[:], in0=nd[:, 0:f], scalar1=zden[:]
            )
            nc.sync.dma_start(out=z_d[:], in_=zout[:])

    return nc


def _prep(inputs):
    x = np.asarray(inputs["x"], dtype=np.float32)
    batch = np.asarray(inputs["batch"]).astype(np.int64)
    gamma = np.asarray(inputs["gamma"], dtype=np.float32)
    beta = np.asarray(inputs["beta"], dtype=np.float32)
    wk = np.asarray(inputs["Wk"], dtype=np.float32)
    wq = np.asarray(inputs["Wq"], dtype=np.float32)
    wv = np.asarray(inputs["Wv"], dtype=np.float32)
    n_batches = int(np.asarray(inputs["n_batches"]))

    n, d = x.shape
    f = wk.shape[0]
    t_seg = n // n_batches
    counts = np.bincount(batch, minlength=n_batches)
    if not (np.all(counts == t_seg) and np.all(np.diff(batch) >= 0)):
        raise NotImplementedError("kernel specialized for equal sorted segments")
    if np.any(beta != 0.0):
        raise NotImplementedError("kernel specialized for beta == 0")

    # LN scale folded into x on the host; centering+gamma folded into W.
    xd = x.astype(np.float64)
    var = xd.var(axis=1)
    r = 1.0 / np.sqrt(var + EPS_LN)
    xs = np.ascontiguousarray((xd * r[:, None]).astype(np.float16))

    wkg = (wk * gamma[None, :]).astype(np.float64)
    wvg = (wv * gamma[None, :]).astype(np.float64)
    wqg = (wq * gamma[None, :]).astype(np.float64)
    wall = np.concatenate([wkg, wvg, wqg], axis=0).T        # [d, 3f]
    wall = wall - wall.sum(axis=0, keepdims=True) / d
    wpackT = np.ascontiguousarray(wall.T.astype(np.float16))  # [3f, d]

    return xs, wpackT, n, d, f, n_batches, t_seg


def _run(inputs, trace=False):
    xs, wpackT, n, d, f, n_batches, t_seg = _prep(inputs)

    segs_per_core = n_batches // N_CORES
    tok_per_core = segs_per_core * t_seg
    nc = _build(tok_per_core, segs_per_core, d, f)

    in_maps = []
    for c in range(N_CORES):
        m = {
            "xs": np.ascontiguousarray(xs[c * tok_per_core:(c + 1) * tok_per_core]),
            "wpackT": wpackT,
        }
        in_maps.append(m)

    res = run_bass_kernel_spmd(nc, in_maps, list(range(N_CORES)), trace=trace)
    z = np.concatenate([res.results[c]["z"] for c in range(N_CORES)], axis=0)
    return z, res


def kernel(**inputs) -> np.ndarray:
    z, _ = _run(inputs, trace=False)
    return z


# revision 36
# speedup vs baseline: 1.1532x; 1.0387x over previous
"""Trainium2 Bass kernel for nn_Encoder_83992380441041 (causal linear attention
encoder, last-position readout).

Math (per segment b of T tokens):
    yn   = LayerNorm(x_b) * gamma + beta          (beta == 0 here)
    K    = phi(yn @ Wk.T); V = yn @ Wv.T; q = phi(yn[T-1] @ Wq.T)
    out  = q @ (K.T V) / (q . sum_t K_t + eps)    [only last position matters]
with phi(a) = elu(a)+1 = min(exp(a),1) + relu(a).

Host folds (all O(N*D) or O(D*F), cheap on CPU):
  * gamma and LN centering into the weights: x @ (W - 1 s~/d) = (x - mu 1) @ W.
  * the LN scale r = 1/sqrt(var+eps) into x itself: ship xs = fp16(r * x).
    Then G = xs @ W' is exactly the phi/V pre-activation; no per-token scale
    or bn_stats on device at all.
Device (per core: 2048 tokens = 8 segments x 256, fp16 data path):
  * xsT [128d x 2048tok] arrives via DMA-transpose (2-byte xbar path).
  * G tile [128tok x 128] = matmul(lhsT=xsT tile, rhs=[Wk'|Wv']).
  * phi: e=Exp(Gk) [ACT], s=relu(Gk) [DVE max], K=min(e,1)+s [DVE stt];
    V copied PSUM->SBUF with ones column appended.
  * S|Z per segment: accumulate matmul(lhsT=K tile, rhs=[V|1]) -> [64 x 65],
    two segments per PSUM bank via tile_position.
  * readout: nd[:,s] = matmul(lhsT=ssb_pair, rhs=qstack col) = [num|q.Z],
    tiny transpose, out = num/(den+eps).

Sharding: data-parallel over segments. 64 segments -> 8 cores x 8 segments.
"""

import os as _os

import numpy as np

import concourse.bass as bass
import concourse.tile as tile
from concourse import mybir
import concourse.bass_utils as _bass_utils
from concourse.bass_utils import run_bass_kernel_spmd
from concourse.vector_clock import ScopedClock
import bass_rust

EPS_LN = 1e-5
EPS_DEN = 1e-5

F32 = mybir.dt.float32
F16 = mybir.dt.float16
AF = mybir.ActivationFunctionType
ALU = mybir.AluOpType

N_CORES = 8


def _patched_drain_and_barrier(self, tick_clock, wait_clock):
    # Stock TileContext exit puts one sem-wait per outstanding proc on a
    # single InstDrain; walrus in this container caps sync waits per
    # instruction. Split them across a chain of drains on the same engine
    # (program order preserved => equivalent).
    nc = self.nc
    drain_inst = nc.sync.drain()
    wait_clock.add_sem_waits(
        drain_inst.ins, ScopedClock({None: tick_clock.global_clock})
    )
    si = drain_inst.ins.sync_info
    if si is not None and si.on_wait is not None and len(si.on_wait) > 1:
        waits = list(si.on_wait)
        si.on_wait = waits[:1]
        for w in waits[1:]:
            d2 = nc.sync.drain()
            si2 = d2.ins.sync_info
            if si2 is None:
                d2.ins.sync_info = bass_rust.SyncInfo(on_wait=[w], on_update=[])
            else:
                si2.on_wait = [w]
    nc.all_engine_barrier()
    assert self.sems is not None
    popped = nc._tile_sem_poison_stack.pop()
    assert popped is self._sem_poison
    nc.clear_and_free_semaphores(list(self.sems.allocated().values()))


tile.TileContext._drain_and_barrier = _patched_drain_and_barrier

_orig_commit = tile.TileContext._commit_instruction
_wsplit_counter = [0]


def _patched_commit_instruction(self, inst, lazy_reg_writes: bool = True):
    # Enforce the per-instruction sync-wait capacity of the walrus in this
    # container (1 for regular instructions, 2 for EventSemaphore) by
    # spilling excess waits onto same-engine NOPs committed just before.
    si = getattr(inst, "sync_info", None)
    if si is not None and si.on_wait:
        cap = 2 if isinstance(inst, mybir.InstEventSemaphore) else 1
        if len(si.on_wait) > cap:
            waits = list(si.on_wait)
            si.on_wait = waits[:cap]
            for w in waits[cap:]:
                _wsplit_counter[0] += 1
                nop = mybir.InstNoOp(
                    name=f"wsplit-{_wsplit_counter[0]}",
                    sync_info=mybir.SyncInfo(on_wait=[w], on_update=[]),
                    bass_nofuse=True,
                    engine=inst.engine,
                )
                _orig_commit(self, nop, lazy_reg_writes=False)
    return _orig_commit(self, inst, lazy_reg_writes=lazy_reg_writes)


tile.TileContext._commit_instruction = _patched_commit_instruction


# Optional extra walrus args (experiment hook), e.g.
#   KERNEL_WALRUS_ARGS="--max-sem-num=32"
_EXTRA_WALRUS_ARGS = [a for a in _os.environ.get("KERNEL_WALRUS_ARGS", "").split() if a]
if _EXTRA_WALRUS_ARGS:
    _orig_bvao = _bass_utils.bir_verify_and_optimise

    def _patched_bvao(tmpdir, inp="bir.json", outp="file.neff", arch=None, *, dve_root=None):
        import subprocess as _sp

        orig_run = _bass_utils.run_command

        def run_with_extra(cmd, **kw):
            return orig_run(list(cmd) + _EXTRA_WALRUS_ARGS, **kw)

        _bass_utils.run_command = run_with_extra
        try:
            return _orig_bvao(tmpdir, inp, outp, arch, dve_root=dve_root)
        finally:
            _bass_utils.run_command = orig_run

    _bass_utils.bir_verify_and_optimise = _patched_bvao


def _build(n_tok: int, n_seg: int, d: int, f: int):
    """Per-core program. Inputs: xs [n_tok,d] fp16 (pre-scaled by r*gamma-fold);
    wkvq [d, 3f] fp16 = [Wk'|Wv'|Wq'] (columns); ident [f+1, f+1] fp16.
    Output: z [n_seg, f] fp32."""
    P = 128
    assert d == P and n_tok % P == 0
    n_tiles = n_tok // P
    t_seg = n_tok // n_seg
    tiles_per_seg = t_seg // P
    assert tiles_per_seg * n_seg == n_tiles
    f2 = 2 * f
    assert f2 == P
    fp1 = f + 1
    B = 4                      # tiles per block (PSUM bank / phi batch)
    n_blk = n_tiles // B
    n_pair = n_seg // 2

    nc = bass.Bass()
    xs_d = nc.declare_dram_parameter("xs", [n_tok, d], F16, isOutput=False)
    # wpackT rows 0..3f-1 = [Wk'|Wv'|Wq'] columns; DMA-transposed on load.
    wpackT_d = nc.declare_dram_parameter("wpackT", [3 * f, d], F16, isOutput=False)
    z_d = nc.declare_dram_parameter("z", [n_seg, f], F32, isOutput=True)

    with tile.TileContext(nc) as tc:
        with (
            tc.tile_pool(name="singles", bufs=1) as singles,
            tc.tile_pool(name="phi", bufs=3) as phip,
            tc.tile_pool(name="ssb", bufs=4) as ssbp,
            tc.tile_pool(name="fin", bufs=1) as finp,
            tc.tile_pool(name="psG", bufs=2, space="PSUM") as psG,
            tc.tile_pool(name="psS", bufs=2, space="PSUM") as psS,
            tc.tile_pool(name="psM", bufs=1, space="PSUM") as psM,
        ):
            # --- persistent buffers ---
            xsT = singles.tile([P, n_tok], F16)
            wkvq = singles.tile([P, 3 * f], F16)
            kbig = singles.tile([P, n_tiles * f], F16)
            v1big = singles.tile([P, n_tiles * fp1], F16)
            jbig = singles.tile([P, 512], F16)

            # --- DMA triggers.  All inputs arrive as DMA-transposes so the
            # tile framework's transpose-vs-DMA serialization edges never
            # fire.  ACT carries the first x half (its only DMA, so the act
            # table load follows right after); SP carries weights + 2nd half.
            half = n_tok // 2
            nc.scalar.dma_start(out=xsT[:, 0:half], in_=xs_d[0:half, :], transpose=True)
            nc.sync.dma_start(out=wkvq[:], in_=wpackT_d[:], transpose=True)
            nc.sync.dma_start(out=xsT[:, half:], in_=xs_d[half:, :], transpose=True)

            # --- PE warm-up junk: no data deps, fills the DMA wait and
            # releases the HAM clock throttle before the real matmuls.
            nc.vector.memset(jbig[:], 0.0)
            for _ in range(4):
                jps = psM.tile([P, 512], F32, tag="m")
                nc.tensor.matmul(
                    jps[:], lhsT=jbig[:, 0:P], rhs=jbig[:],
                    start=True, stop=True, skip_group_check=True,
                )

            # ones columns of [V|1] (stride fp1 within v1big)
            nc.vector.memset(v1big[:, f::fp1], 1.0)

            # qpad: 4 zero-padded copies of qstack, one per segment pair —
            # pair p's slice has only columns 2p,2p+1 nonzero, so the four
            # readout matmuls can accumulate into one [n_seg, f+1] group.
            qpad = finp.tile([P, n_pair * n_seg], F16)
            zden = finp.tile([n_seg, 1], F32)
            zout = finp.tile([n_seg, f], F32)
            eq = finp.tile([P, n_seg], F16)
            sq = finp.tile([P, n_seg], F16)
            q2 = finp.tile([P, n_seg], F16)

            wkv_s = wkvq[:, 0:f2]
            wq_s = wkvq[:, f2:3 * f]

            # --- per-block (4 tiles = 1 segment pair): G -> batched phi /
            # V copy -> S -> ssb copy.  V/ssb copies alternate ACT and DVE
            # to balance the two elementwise engines.
            ssbs = []
            for b in range(n_blk):
                b0 = b * B
                gps = psG.tile([P, B * P], F32)
                for j in range(B):
                    nc.tensor.matmul(
                        gps[:, j * P:(j + 1) * P],
                        lhsT=xsT[:, (b0 + j) * P:(b0 + j + 1) * P],
                        rhs=wkv_s,
                        start=True, stop=True, skip_group_check=True,
                    )
                blk = gps[:].rearrange("p (j c) -> p j c", c=P)
                gk = blk[:, :, 0:f]
                gv = blk[:, :, f:f2]
                e_t = phip.tile([P, B * f], F16, tag="e")
                nc.scalar.activation(out=e_t[:], in_=gk, func=AF.Exp)
                s_t = phip.tile([P, B * f], F16, tag="s")
                nc.vector.tensor_scalar_max(out=s_t[:], in0=gk, scalar1=0.0)
                nc.vector.scalar_tensor_tensor(
                    out=kbig[:, b0 * f:(b0 + B) * f], in0=e_t[:],
                    scalar=1.0, in1=s_t[:],
                    op0=ALU.min, op1=ALU.add,
                )
                vdst = v1big[:, b0 * fp1:(b0 + B) * fp1].rearrange(
                    "p (j c) -> p j c", c=fp1
                )[:, :, 0:f]
                if b % 2 == 0:
                    nc.scalar.activation(out=vdst, in_=gv, func=AF.Copy)
                else:
                    nc.vector.tensor_copy(out=vdst, in_=gv)

                # S|Z for the block's two segments
                assert B == 2 * tiles_per_seg
                sps = psS.tile([P, fp1], F32)
                for hh in range(2):
                    s = 2 * b + hh
                    for j in range(tiles_per_seg):
                        n = s * tiles_per_seg + j
                        nc.tensor.matmul(
                            sps[hh * f:(hh + 1) * f, :],
                            lhsT=kbig[:, n * f:(n + 1) * f],
                            rhs=v1big[:, n * fp1:(n + 1) * fp1],
                            start=(j == 0), stop=(j == tiles_per_seg - 1),
                            skip_group_check=True,
                            tile_position=(0, hh * f),
                        )
                ssb = ssbp.tile([P, fp1], F16)
                if b < n_blk - 1:
                    nc.scalar.activation(out=ssb[:], in_=sps[:], func=AF.Copy)
                else:
                    nc.vector.tensor_copy(out=ssb[:], in_=sps[:])
                ssbs.append(ssb)

            # --- q path: last token of each segment, both PE-halves at once
            xq = xsT[:, t_seg - 1::t_seg]
            qc = psM.tile([P, n_seg], F32, tag="m")
            nc.tensor.matmul(
                qc[0:f, :], lhsT=wq_s, rhs=xq,
                start=True, stop=True, skip_group_check=True,
            )
            nc.tensor.matmul(
                qc[f:f2, :], lhsT=wq_s, rhs=xq,
                start=True, stop=True, skip_group_check=True,
                tile_position=(0, f),
            )
            nc.scalar.activation(out=eq[:], in_=qc[:], func=AF.Exp)
            nc.vector.tensor_scalar_max(out=sq[:], in0=qc[:], scalar1=0.0)
            nc.vector.scalar_tensor_tensor(
                out=q2[:], in0=eq[:], scalar=1.0, in1=sq[:],
                op0=ALU.min, op1=ALU.add,
            )
            # pair p's even segment lives at global col 10p, odd at 10p+1
            nc.vector.memset(qpad[:], 0.0)
            nc.vector.tensor_copy(
                out=qpad[0:f, 0::2 + n_seg], in_=q2[0:f, 0:n_seg:2]
            )
            nc.vector.tensor_copy(
                out=qpad[f:f2, 1::2 + n_seg], in_=q2[f:f2, 1:n_seg:2]
            )

            # --- readout, accumulated across pairs:
            #   nd[s, :] = qhat_s^T @ ssb_pair(s) = [num_s | q_s.Z_s]
            nd = psM.tile([n_seg, fp1], F32, tag="nd")
            for p in range(n_pair):
                nc.tensor.matmul(
                    nd[:], lhsT=qpad[:, p * n_seg:(p + 1) * n_seg],
                    rhs=ssbs[p],
                    start=(p == 0), stop=(p == n_pair - 1),
                    skip_group_check=True,
                )
            nc.vector.tensor_scalar_add(
                out=zden[:], in0=nd[:, f:fp1], scalar1=EPS_DEN
            )
            nc.vector.reciprocal(out=zden[:], in_=zden[:])
            nc.vector.tensor_scalar_mul(
                out=zout[:], in0=nd[:, 0:f], scalar1=zden[:]
            )
            nc.sync.dma_start(out=z_d[:], in_=zout[:])

    return nc


def _prep(inputs):
    x = np.asarray(inputs["x"], dtype=np.float32)
    batch = np.asarray(inputs["batch"]).astype(np.int64)
    gamma = np.asarray(inputs["gamma"], dtype=np.float32)
    beta = np.asarray(inputs["beta"], dtype=np.float32)
    wk = np.asarray(inputs["Wk"], dtype=np.float32)
    wq = np.asarray(inputs["Wq"], dtype=np.float32)
    wv = np.asarray(inputs["Wv"], dtype=np.float32)
    n_batches = int(np.asarray(inputs["n_batches"]))

    n, d = x.shape
    f = wk.shape[0]
    t_seg = n // n_batches
    counts = np.bincount(batch, minlength=n_batches)
    if not (np.all(counts == t_seg) and np.all(np.diff(batch) >= 0)):
        raise NotImplementedError("kernel specialized for equal sorted segments")
    if np.any(beta != 0.0):
        raise NotImplementedError("kernel specialized for beta == 0")

    # LN scale folded into x on the host; centering+gamma folded into W.
    xd = x.astype(np.float64)
    var = xd.var(axis=1)
    r = 1.0 / np.sqrt(var + EPS_LN)
    xs = np.ascontiguousarray((xd * r[:, None]).astype(np.float16))

    wkg = (wk * gamma[None, :]).astype(np.float64)
    wvg = (wv * gamma[None, :]).astype(np.float64)
    wqg = (wq * gamma[None, :]).astype(np.float64)
    wall = np.concatenate([wkg, wvg, wqg], axis=0).T        # [d, 3f]
    wall = wall - wall.sum(axis=0, keepdims=True) / d
    wpackT = np.ascontiguousarray(wall.T.astype(np.float16))  # [3f, d]

    return xs, wpackT, n, d, f, n_batches, t_seg


def _run(inputs, trace=False):
    xs, wpackT, n, d, f, n_batches, t_seg = _prep(inputs)

    segs_per_core = n_batches // N_CORES
    tok_per_core = segs_per_core * t_seg
    nc = _build(tok_per_core, segs_per_core, d, f)

    in_maps = []
    for c in range(N_CORES):
        m = {
            "xs": np.ascontiguousarray(xs[c * tok_per_core:(c + 1) * tok_per_core]),
            "wpackT": wpackT,
        }
        in_maps.append(m)

    res = run_bass_kernel_spmd(nc, in_maps, list(range(N_CORES)), trace=trace)
    z = np.concatenate([res.results[c]["z"] for c in range(N_CORES)], axis=0)
    return z, res


def kernel(**inputs) -> np.ndarray:
    z, _ = _run(inputs, trace=False)
    return z
